# revision 20
# baseline (speedup 1.0000x reference)
"""2-layer GCN + global mean-pool + MLP head on 8 TRN2 NeuronCores.

Algorithm (matches the PyG-style reference):
    h1 = relu(Ahat @ (x @ W1) + b1)
    h2 = relu(Ahat @ (h1 @ W2) + b2)
    g  = segment_mean(h2, batch)          # [64, hid]
    out = relu(g @ Wc1 + bc1) @ Wc2 + bc2 # [64, 1]
with Ahat = D^-1/2 (A + I) D^-1/2 (in-degree based, self-loops added).

Key factorization: the edge weight dinv[s]*dinv[d] splits per-node, so we
pre-scale "table" rows by dinv (T = dinv * (h @ W)) and aggregation becomes a
pure gather+sum followed by a per-destination dinv scale:
    out[d] = dinv[d] * sum_{s in N(d) ∪ {d}} T[s]  + b

Distribution: nodes are permuted so that similar-(in)degree nodes share a
128-row tile; 49 tiles per core x 8 cores = 50176 padded rows.  Each core:
  1. computes T1 for its rows via TensorE (x pre-scaled/transposed on host),
  2. AllGather -> full T1 table in every core's HBM,
  3. per tile group: bulk dma_gather of all slot rows (slot-major layout),
     DVE sums over slots, scale/bias/relu, matmul with W2 -> T2 shard,
  4. AllGather T2, second aggregate pass, pooling matmul into PSUM,
  5. AllReduce pooled [hid, graphs] partials, replicated tiny MLP head.

dma_gather (InstDMAGatherAnt) takes int16 indices, so the 50176-row table is
addressed through two overlapping 32768-row windows ([0, 32768) and
[17408, 50176)); each node's slots are bucketed per window (rows in the
overlap go to whichever bucket balances the two), padded per tile to the
group bucket max, with pad slots rotating over dummy rows in the overlap
region (T value exactly 0, and spread to avoid HBM hot lines).  TGROUP tiles
share one gather instruction per window to amortize per-instruction cost,
and instructions round-robin over 4 SWDGE queues.
"""

import numpy as np

import concourse.bass as bass
import concourse.mybir as mybir
import concourse.tile as tile
from concourse import bacc
from concourse.bass_utils import run_bass_kernel_spmd
from concourse.masks import make_identity

NCORES = 8
P = 128
N = 50000
G = 64
IN_DIM = 128
TPC = 49                 # node tiles per core
NPC = TPC * P            # 6272 rows per core
NTOT = NPC * NCORES      # 50176
WROW = 32768             # int16 index window width
WBASE = (0, NTOT - WROW)  # two overlapping gather windows
TGROUP = 4               # tiles per gather instruction
ALIGN = 256              # idx block column alignment (x2B = 512B)

F32 = mybir.dt.float32
F16 = mybir.dt.float16
I16 = mybir.dt.int16
TDT = F32                # gather-table dtype: dma_gather needs 256B rows -> f32
ADT = F16                # activation/weight dtype (PE inputs, pool matmul)
NPT = np.float16


def _pad_to(x, a):
    return (x + a - 1) // a * a


def _groups():
    return [list(range(g, min(g + TGROUP, TPC))) for g in range(0, TPC, TGROUP)]


def _pack16(vals):
    """int16 index layout consumed by the Q7 dma_gather kernel:
    sbuf[p, i] = vals[i*16 + p%16], replicated across the 8 partition groups."""
    m = len(vals) // 16
    a = vals.astype(np.int16).reshape(m, 16).T      # [16, m]
    return np.tile(a, (8, 1))                        # [128, m]


def prep(x, edge_index, batch, W1, b1, W2, b2, Wc1, bc1, Wc2, bc2):
    """Host-side graph preprocessing -> (DG, in_maps).

    DG[w][gi]: per-window slots per tile for gather group gi (uniform over
    the group's tiles and all cores).
    """
    x = np.ascontiguousarray(np.asarray(x, dtype=np.float32))
    ei = np.asarray(edge_index).astype(np.int64)
    batch = np.asarray(batch).astype(np.int64)
    src, dst = ei[0], ei[1]

    indeg = np.bincount(dst, minlength=N)
    d_slots = (indeg + 1).astype(np.int64)          # incident slots incl. self
    dinv = (1.0 / np.sqrt(d_slots.astype(np.float32))).astype(np.float32)

    # Sort nodes by slot count desc; tiles of 128 consecutive sorted nodes
    # have near-uniform degree.  Tile t -> core t%8, tile position t//8.
    order = np.argsort(-d_slots, kind="stable")
    pos = np.arange(NTOT)
    t_idx = pos // P
    newrow = (t_idx % NCORES) * NPC + (t_idx // NCORES) * P + pos % P
    # Dummy (padding) sorted positions are N..NTOT-1.  Gather padding targets
    # dummy rows (their T value is exactly 0).  Move every dummy into the
    # region covered by both windows ([17408, 32768)) by swapping with
    # similar-(low-)degree real nodes there, and spread pads over all of them
    # so pad reads don't hot-spot a single HBM line.
    ndum = NTOT - N
    dummy_pos = np.arange(N, NTOT)
    in_overlap = (newrow >= WBASE[1]) & (newrow < WROW)
    cand = np.where(in_overlap[:N])[0][-ndum:]    # lowest-degree reals there
    assert len(cand) == ndum
    tmp = newrow[cand].copy()
    newrow[cand] = newrow[dummy_pos]
    newrow[dummy_pos] = tmp
    newid = np.empty(N, dtype=np.int64)
    newid[order] = newrow[:N]
    zrows = np.sort(newrow[N:])
    assert (zrows >= WBASE[1]).all() and (zrows < WROW).all()

    # Edge list in new ids, with self-loops appended, sorted by destination.
    src_new = newid[src]
    dst_new = newid[dst]
    S_all = np.concatenate([src_new, np.arange(NTOT, dtype=np.int64)])
    T_all = np.concatenate([dst_new, np.arange(NTOT, dtype=np.int64)])
    order_e = np.argsort(T_all, kind="stable")
    S = S_all[order_e]
    T = T_all[order_e]
    E2 = len(S)
    counts = np.bincount(T_all, minlength=NTOT)
    starts = np.zeros(NTOT + 1, dtype=np.int64)
    np.cumsum(counts, out=starts[1:])
    T_start = starts[T]

    # Window assignment: rows < 17408 must use window 0, rows >= 32768 must
    # use window 1; rows in the overlap go to whichever bucket balances the
    # destination's two counts (padded per-tile max is what descriptors cost).
    forced_lo = S < WBASE[1]
    forced_hi = S >= WROW
    flex = ~forced_lo & ~forced_hi
    a_cnt = np.bincount(T[forced_lo], minlength=NTOT)
    c_cnt = np.bincount(T[flex], minlength=NTOT)
    x_cnt = np.clip((counts + 1) // 2 - a_cnt, 0, c_cnt)   # flex edges -> lo
    excl_f = np.cumsum(flex) - flex
    flex_rank = excl_f - excl_f[T_start]
    is_lo = forced_lo | (flex & (flex_rank < x_cnt[T]))
    excl = np.cumsum(is_lo) - is_lo
    lo_rank = excl - excl[T_start]
    hi_rank = (np.arange(E2) - T_start) - lo_rank
    wcnt = [np.bincount(T[is_lo], minlength=NTOT)]
    wcnt.append(counts - wcnt[0])
    wrank = [lo_rank, hi_rank]
    wsel = [is_lo, ~is_lo]

    # per-group per-window uniform slot counts (max over group tiles x cores)
    groups = _groups()
    per_tile = [w.reshape(NCORES, TPC, P).max(axis=(0, 2)) for w in wcnt]
    DG = [[int(per_tile[w][tiles].max()) for tiles in groups] for w in range(2)]

    mats = []
    ii = np.arange(NTOT)[:, None]
    for w in range(2):
        dmax = max(1, max(DG[w]))
        jw = np.arange(dmax)[None, :]
        m = (zrows[(ii * 7 + jw) % len(zrows)] - WBASE[w]).astype(np.int32)
        m[T[wsel[w]], wrank[w][wsel[w]]] = S[wsel[w]] - WBASE[w]
        mats.append(m)

    # Permuted per-node data
    dinv_new = np.zeros(NTOT, dtype=np.float32)
    dinv_new[newid] = dinv
    x_new = np.zeros((NTOT, IN_DIM), dtype=np.float32)
    x_new[newid] = x * dinv[:, None]                 # pre-scaled by dinv
    cnts = np.bincount(batch, minlength=G).astype(np.float32)
    inv_cnt = 1.0 / np.maximum(cnts, 1.0)
    pool_new = np.zeros((NTOT, G), dtype=np.float32)
    pool_new[newid, batch] = inv_cnt[batch]

    W1 = np.ascontiguousarray(np.asarray(W1, NPT))
    W2 = np.ascontiguousarray(np.asarray(W2, NPT))
    Wc1 = np.ascontiguousarray(np.asarray(Wc1, np.float32))
    Wc2 = np.ascontiguousarray(np.asarray(Wc2, np.float32))
    b1r = np.ascontiguousarray(np.broadcast_to(np.asarray(b1, np.float32), (P, 64)))
    b2r = np.ascontiguousarray(np.broadcast_to(np.asarray(b2, np.float32), (P, 64)))
    bc1c = np.ascontiguousarray(np.asarray(bc1, np.float32).reshape(32, 1))
    bc2c = np.ascontiguousarray(np.asarray(bc2, np.float32).reshape(1, 1))

    in_maps = []
    for c in range(NCORES):
        rows = slice(c * NPC, (c + 1) * NPC)
        blocks = []
        for gi, tiles in enumerate(groups):
            for w in range(2):
                D = DG[w][gi]
                vals = np.concatenate(
                    [
                        mats[w][c * NPC + k * P : c * NPC + (k + 1) * P, :D].T.ravel()
                        for k in tiles
                    ]
                )
                b = _pack16(vals)                     # [128, len(tiles)*8*D]
                padc = _pad_to(b.shape[1], ALIGN) - b.shape[1]
                if padc:
                    b = np.concatenate([b, np.zeros((P, padc), np.int16)], axis=1)
                blocks.append(b)
        idx16_c = np.ascontiguousarray(np.concatenate(blocks, axis=1))
        in_maps.append(
            {
                "xt": np.ascontiguousarray(x_new[rows].T.astype(NPT)),  # [128, NPC]
                "idx16": idx16_c,                                # [128, IDXW]
                "dinv": np.ascontiguousarray(dinv_new[rows]),    # [NPC]
                "pool": np.ascontiguousarray(pool_new[rows].astype(NPT)),  # [NPC, G]
                "w1": W1,
                "w2": W2,
                "wc1": Wc1,
                "wc2": Wc2,
                "b1r": b1r,
                "b2r": b2r,
                "bc1": bc1c,
                "bc2": bc2c,
            }
        )
    return DG, in_maps


def build_nc(DG, debug=False, nrep=1, nq=4, table_mode="shared", scratch=32768):
    """Build the SPMD Bass program (shared by all 8 cores)."""
    assert not (debug and nrep > 1)
    groups = _groups()
    idx_off = []
    off = 0
    for gi, tiles in enumerate(groups):
        offs = []
        for w in range(2):
            offs.append(off)
            off += _pad_to(len(tiles) * 8 * DG[w][gi], ALIGN)
        idx_off.append(offs)
    idxw = off

    nc = bacc.Bacc(num_swdge_queues=nq, dynamic_dma_scratch_size=scratch)

    xt = nc.declare_dram_parameter("xt", [IN_DIM, NPC], ADT, isOutput=False)
    idx16 = nc.declare_dram_parameter("idx16", [P, idxw], I16, isOutput=False)
    dinv = nc.declare_dram_parameter("dinv", [NPC], F32, isOutput=False)
    pool = nc.declare_dram_parameter("pool", [NPC, G], ADT, isOutput=False)
    w1 = nc.declare_dram_parameter("w1", [IN_DIM, 64], ADT, isOutput=False)
    w2 = nc.declare_dram_parameter("w2", [64, 64], ADT, isOutput=False)
    wc1 = nc.declare_dram_parameter("wc1", [64, 32], F32, isOutput=False)
    wc2 = nc.declare_dram_parameter("wc2", [32, 1], F32, isOutput=False)
    b1r = nc.declare_dram_parameter("b1r", [P, 64], F32, isOutput=False)
    b2r = nc.declare_dram_parameter("b2r", [P, 64], F32, isOutput=False)
    bc1 = nc.declare_dram_parameter("bc1", [32, 1], F32, isOutput=False)
    bc2 = nc.declare_dram_parameter("bc2", [1, 1], F32, isOutput=False)
    out_ext = nc.declare_dram_parameter("out", [1, G], F32, isOutput=True)

    t1_shard = nc.dram_tensor("t1_shard", [NPC, 64], TDT)
    t2_shard = nc.dram_tensor("t2_shard", [NPC, 64], TDT)
    out_space = "Local" if table_mode == "local_out" else "Shared"
    t1_full = nc.dram_tensor("t1_full", [NTOT, 64], TDT, addr_space=out_space)
    t2_full = nc.dram_tensor("t2_full", [NTOT, 64], TDT, addr_space=out_space)
    gpart = nc.dram_tensor("gpart", [64, G], F32)
    gfull = nc.dram_tensor("gfull", [64, G], F32, addr_space="Shared")

    rg = [list(range(NCORES))]
    relu = mybir.ActivationFunctionType.Relu
    ident_f = mybir.ActivationFunctionType.Identity

    with tile.TileContext(nc) as tc:
        with (
            tc.tile_pool(name="singles", bufs=1) as singles,
            tc.tile_pool(name="resident", bufs=1) as resident,
            tc.tile_pool(name="work", bufs=6) as work,
            tc.tile_pool(name="gath", bufs=3) as gath,
            tc.tile_pool(name="psum", bufs=2, space="PSUM") as psum,
            tc.tile_pool(name="psum1", bufs=1, space="PSUM") as psum1,
        ):
            # --- constants ---
            xt_sb = singles.tile([P, NPC], ADT)
            nc.sync.dma_start(out=xt_sb[:], in_=xt[:])
            idx_sb = singles.tile([P, idxw], I16)
            nc.sync.dma_start(out=idx_sb[:], in_=idx16[:])
            w1_sb = singles.tile([IN_DIM, 64], ADT)
            nc.sync.dma_start(out=w1_sb[:], in_=w1[:])
            w2_sb = singles.tile([64, 64], ADT)
            nc.sync.dma_start(out=w2_sb[:], in_=w2[:])
            wc1_sb = singles.tile([64, 32], F32)
            nc.sync.dma_start(out=wc1_sb[:], in_=wc1[:])
            wc2_sb = singles.tile([32, 1], F32)
            nc.sync.dma_start(out=wc2_sb[:], in_=wc2[:])
            b1r_sb = singles.tile([P, 64], F32)
            nc.sync.dma_start(out=b1r_sb[:], in_=b1r[:])
            b2r_sb = singles.tile([P, 64], F32)
            nc.sync.dma_start(out=b2r_sb[:], in_=b2r[:])
            bc1_sb = singles.tile([32, 1], F32)
            nc.sync.dma_start(out=bc1_sb[:], in_=bc1[:])
            bc2_sb = singles.tile([1, 1], F32)
            nc.sync.dma_start(out=bc2_sb[:], in_=bc2[:])
            ident_sb = singles.tile([P, P], ADT)
            make_identity(nc, ident_sb[:])

            dinv_sb = []
            pool_sb = []
            for k in range(TPC):
                dt = resident.tile([P, 1], F32, tag=f"dinv{k}")
                nc.sync.dma_start(out=dt[:], in_=dinv[k * P : (k + 1) * P, None])
                dinv_sb.append(dt)
                pt = resident.tile([P, G], ADT, tag=f"pool{k}")
                nc.sync.dma_start(out=pt[:], in_=pool[k * P : (k + 1) * P, :])
                pool_sb.append(pt)

            gcnt = [0]  # global SWDGE gather counter -> DMASW lane rotation

            def gather_group(g_t, table, gi, ntile):
                """Two gathers (one per window) covering the whole group."""
                col = 0
                for w in range(2):
                    n = ntile * DG[w][gi]
                    if n == 0:
                        continue
                    o = idx_off[gi][w]
                    # Tile locks each DMASW sem lane (8, round-robin in
                    # creation order) to a single SWDGE queue; follow that.
                    q = (gcnt[0] % 8) // (8 // nq)
                    gcnt[0] += 1
                    nc.gpsimd.dma_gather(
                        out_ap=g_t[:, col * 64 : (col + n) * 64].rearrange(
                            "p (c f) -> p c f", f=64
                        ),
                        in_ap=table[WBASE[w] : WBASE[w] + WROW, :],
                        idxs_ap=idx_sb[:, o : o + 8 * n],
                        num_idxs=P * n,
                        num_idxs_reg=P * n,
                        elem_size=64,
                        single_packet=False,
                        queue_num=q,
                    )
                    col += n

            def aggregate(table, bias_sb):
                """Gather+reduce+scale+bias for one layer; yields (k, pre).

                One tensor_reduce per (group, window) via a 4D AP
                [p, tile, feat, slot] reducing the innermost slot axis."""
                for gi, tiles in enumerate(groups):
                    ntile = len(tiles)
                    D0, D1 = DG[0][gi], DG[1][gi]
                    g_t = gath.tile([P, ntile * (D0 + D1) * 64], TDT, tag="g")
                    gather_group(g_t, table, gi, ntile)
                    s_t = work.tile([P, ntile * 64], F32, tag="s")
                    nc.vector.tensor_reduce(
                        out=s_t[:].rearrange("p (t f) -> p t f", f=64),
                        in_=g_t[:, : ntile * D0 * 64].rearrange(
                            "p (t d f) -> p t f d", d=D0, f=64
                        ),
                        axis=mybir.AxisListType.X,
                        op=mybir.AluOpType.add,
                    )
                    if D1:
                        s2_t = work.tile([P, ntile * 64], F32, tag="s2")
                        nc.vector.tensor_reduce(
                            out=s2_t[:].rearrange("p (t f) -> p t f", f=64),
                            in_=g_t[:, ntile * D0 * 64 :].rearrange(
                                "p (t d f) -> p t f d", d=D1, f=64
                            ),
                            axis=mybir.AxisListType.X,
                            op=mybir.AluOpType.add,
                        )
                        nc.vector.tensor_add(out=s_t[:], in0=s_t[:], in1=s2_t[:])
                    for j, k in enumerate(tiles):
                        pre_t = work.tile([P, 64], F32, tag="pre")
                        nc.vector.scalar_tensor_tensor(
                            out=pre_t[:],
                            in0=s_t[:, j * 64 : (j + 1) * 64],
                            scalar=dinv_sb[k][:],
                            in1=bias_sb[:],
                            op0=mybir.AluOpType.mult,
                            op1=mybir.AluOpType.add,
                        )
                        yield k, pre_t

            for _rep in range(nrep):
                # --- phase B: T1 shard = (x*dinv) @ W1 ---
                for k in range(TPC):
                    ps = psum.tile([P, 64], F32, tag="mm")
                    nc.tensor.matmul(
                        out=ps[:],
                        lhsT=xt_sb[:, k * P : (k + 1) * P],
                        rhs=w1_sb[:],
                        start=True,
                        stop=True,
                    )
                    t1t = work.tile([P, 64], TDT, tag="t1t")
                    nc.scalar.copy(out=t1t[:], in_=ps[:])
                    nc.sync.dma_start(out=t1_shard[k * P : (k + 1) * P, :], in_=t1t[:])

                # --- phase C: AllGather T1 ---
                nc.gpsimd.collective_compute(
                    "AllGather",
                    mybir.AluOpType.bypass,
                    replica_groups=rg,
                    ins=[t1_shard[:]],
                    outs=[t1_full[:]],
                )

                # --- phase D: layer-1 aggregate + T2 shard ---
                for k, pre_t in aggregate(t1_full, b1r_sb):
                    # a1_scaled = dinv*relu(pre) = relu(dinv*pre)   (dinv >= 0)
                    a1s_t = work.tile([P, 64], ADT, tag="a1s")
                    nc.scalar.activation(
                        out=a1s_t[:], in_=pre_t[:], func=relu, scale=dinv_sb[k][:]
                    )
                    pst = psum.tile([64, P], ADT, tag="tr")
                    nc.tensor.transpose(out=pst[:], in_=a1s_t[:], identity=ident_sb[:])
                    a1sT = work.tile([64, P], ADT, tag="a1sT")
                    nc.scalar.copy(out=a1sT[:], in_=pst[:])
                    ps2 = psum.tile([P, 64], F32, tag="mm")
                    nc.tensor.matmul(
                        out=ps2[:], lhsT=a1sT[:], rhs=w2_sb[:], start=True, stop=True
                    )
                    t2t = work.tile([P, 64], TDT, tag="t2t")
                    nc.scalar.copy(out=t2t[:], in_=ps2[:])
                    nc.sync.dma_start(out=t2_shard[k * P : (k + 1) * P, :], in_=t2t[:])

                # --- phase E: AllGather T2 ---
                nc.gpsimd.collective_compute(
                    "AllGather",
                    mybir.AluOpType.bypass,
                    replica_groups=rg,
                    ins=[t2_shard[:]],
                    outs=[t2_full[:]],
                )

                # --- phase F: layer-2 aggregate + pooling matmul ---
                pool_ps = psum1.tile([64, G], F32)
                for k, pre_t in aggregate(t2_full, b2r_sb):
                    a2_t = work.tile([P, 64], ADT, tag="a2")
                    nc.scalar.activation(out=a2_t[:], in_=pre_t[:], func=relu)
                    nc.tensor.matmul(
                        out=pool_ps[:],
                        lhsT=a2_t[:],
                        rhs=pool_sb[k][:],
                        start=(k == 0),
                        stop=(k == TPC - 1),
                    )

                # --- phase G: AllReduce pooled partials ---
                gpart_sb = work.tile([64, G], F32, tag="gp")
                nc.scalar.copy(out=gpart_sb[:], in_=pool_ps[:])
                nc.sync.dma_start(out=gpart[:], in_=gpart_sb[:])
                nc.gpsimd.collective_compute(
                    "AllReduce",
                    mybir.AluOpType.add,
                    replica_groups=rg,
                    ins=[gpart[:]],
                    outs=[gfull[:]],
                )
                gsum_sb = work.tile([64, G], F32, tag="gs")
                nc.sync.dma_start(out=gsum_sb[:], in_=gfull[:])

                # --- phase H: MLP head (replicated) ---
                ps_z = psum1.tile([32, G], F32, tag="hz")
                nc.tensor.matmul(
                    out=ps_z[:], lhsT=wc1_sb[:], rhs=gsum_sb[:], start=True, stop=True
                )
                z_sb = work.tile([32, G], F32, tag="z")
                nc.scalar.activation(out=z_sb[:], in_=ps_z[:], func=relu, bias=bc1_sb[:])
                ps_o = psum1.tile([1, G], F32, tag="ho")
                nc.tensor.matmul(
                    out=ps_o[:], lhsT=wc2_sb[:], rhs=z_sb[:], start=True, stop=True
                )
                o_sb = work.tile([1, G], F32, tag="o")
                nc.scalar.activation(
                    out=o_sb[:], in_=ps_o[:], func=ident_f, bias=bc2_sb[:]
                )
                nc.sync.dma_start(out=out_ext[:], in_=o_sb[:])

            if debug:
                t1_dump = nc.declare_dram_parameter(
                    "t1_dump", [NTOT, 64], TDT, isOutput=True
                )
                t2_dump = nc.declare_dram_parameter(
                    "t2_dump", [NTOT, 64], TDT, isOutput=True
                )
                nc.sync.dma_start(out=t1_dump[:], in_=t1_full[:])
                nc.sync.dma_start(out=t2_dump[:], in_=t2_full[:])

    nc.finalize()
    return nc


def kernel(**inputs):
    DG, in_maps = prep(**inputs)
    nc = build_nc(DG)
    res = run_bass_kernel_spmd(nc, in_maps, list(range(NCORES)))
    out = np.asarray(res.results[0]["out"], dtype=np.float32).reshape(G, 1)
    return out



# revision 23
# speedup vs baseline: 4.4383x; 4.4383x over previous
"""2-layer GCN + global mean-pool + MLP head on 8 TRN2 NeuronCores.

Algorithm (matches the PyG-style reference):
    h1 = relu(Ahat @ (x @ W1) + b1)
    h2 = relu(Ahat @ (h1 @ W2) + b2)
    g  = segment_mean(h2, batch)          # [64, hid]
    out = relu(g @ Wc1 + bc1) @ Wc2 + bc2 # [64, 1]
with Ahat = D^-1/2 (A + I) D^-1/2 (in-degree based, self-loops added).

Key factorization: the edge weight dinv[s]*dinv[d] splits per-node, so we
pre-scale "table" rows by dinv (T = dinv * (h @ W)) and aggregation becomes a
pure gather+sum followed by a per-destination dinv scale:
    out[d] = dinv[d] * sum_{s in N(d) ∪ {d}} T[s]  + b

Distribution: nodes are permuted so that similar-(in)degree nodes share a
128-row tile; 49 tiles per core x 8 cores = 50176 padded rows.  Each core:
  1. computes T1 for its rows via TensorE (x pre-scaled/transposed on host),
  2. AllGather -> full T1 table in every core's HBM,
  3. per tile group: bulk dma_gather of all slot rows (slot-major layout),
     DVE sums over slots, scale/bias/relu, matmul with W2 -> T2 shard,
  4. AllGather T2, second aggregate pass, pooling matmul into PSUM,
  5. AllReduce pooled [hid, graphs] partials, replicated tiny MLP head.

dma_gather (InstDMAGatherAnt) takes int16 indices, so the 50176-row table is
addressed through two overlapping 32768-row windows ([0, 32768) and
[17408, 50176)); each node's slots are bucketed per window (rows in the
overlap go to whichever bucket balances the two), padded per tile to the
group bucket max, with pad slots rotating over dummy rows in the overlap
region (T value exactly 0, and spread to avoid HBM hot lines).  TGROUP tiles
share one gather instruction per window to amortize per-instruction cost,
and instructions round-robin over 4 SWDGE queues.
"""

import numpy as np

import concourse.bass as bass
import concourse.mybir as mybir
import concourse.tile as tile
from concourse import bacc
from concourse.bass_utils import run_bass_kernel_spmd
from concourse.masks import make_identity

NCORES = 8
P = 128
N = 50000
G = 64
IN_DIM = 128
TPC = 49                 # node tiles per core
NPC = TPC * P            # 6272 rows per core
NTOT = NPC * NCORES      # 50176
WROW = 32768             # int16 index window width
WBASE = (0, NTOT - WROW)  # two overlapping gather windows
TGROUP = 4               # tiles per gather instruction
ALIGN = 256              # idx block column alignment (x2B = 512B)

F32 = mybir.dt.float32
F16 = mybir.dt.float16
I16 = mybir.dt.int16
TDT = F32                # gather-table dtype: dma_gather needs 256B rows -> f32
ADT = F16                # activation/weight dtype (PE inputs, pool matmul)
NPT = np.float16


def _pad_to(x, a):
    return (x + a - 1) // a * a


def _groups():
    return [list(range(g, min(g + TGROUP, TPC))) for g in range(0, TPC, TGROUP)]


def _pack16(vals):
    """int16 index layout consumed by the Q7 dma_gather kernel:
    sbuf[p, i] = vals[i*16 + p%16], replicated across the 8 partition groups."""
    m = len(vals) // 16
    a = vals.astype(np.int16).reshape(m, 16).T      # [16, m]
    return np.tile(a, (8, 1))                        # [128, m]


def prep(x, edge_index, batch, W1, b1, W2, b2, Wc1, bc1, Wc2, bc2):
    """Host-side graph preprocessing -> (DG, in_maps).

    DG[w][gi]: per-window slots per tile for gather group gi (uniform over
    the group's tiles and all cores).
    """
    x = np.ascontiguousarray(np.asarray(x, dtype=np.float32))
    ei = np.asarray(edge_index).astype(np.int64)
    batch = np.asarray(batch).astype(np.int64)
    src, dst = ei[0], ei[1]

    indeg = np.bincount(dst, minlength=N)
    d_slots = (indeg + 1).astype(np.int64)          # incident slots incl. self
    dinv = (1.0 / np.sqrt(d_slots.astype(np.float32))).astype(np.float32)

    # Sort nodes by slot count desc; tiles of 128 consecutive sorted nodes
    # have near-uniform degree.  Tile t -> core t%8, tile position t//8.
    order = np.argsort(-d_slots, kind="stable")
    pos = np.arange(NTOT)
    t_idx = pos // P
    newrow = (t_idx % NCORES) * NPC + (t_idx // NCORES) * P + pos % P
    # Dummy (padding) sorted positions are N..NTOT-1.  Gather padding targets
    # dummy rows (their T value is exactly 0).  Move every dummy into the
    # region covered by both windows ([17408, 32768)) by swapping with
    # similar-(low-)degree real nodes there, and spread pads over all of them
    # so pad reads don't hot-spot a single HBM line.
    ndum = NTOT - N
    dummy_pos = np.arange(N, NTOT)
    in_overlap = (newrow >= WBASE[1]) & (newrow < WROW)
    cand = np.where(in_overlap[:N])[0][-ndum:]    # lowest-degree reals there
    assert len(cand) == ndum
    tmp = newrow[cand].copy()
    newrow[cand] = newrow[dummy_pos]
    newrow[dummy_pos] = tmp
    newid = np.empty(N, dtype=np.int64)
    newid[order] = newrow[:N]
    zrows = np.sort(newrow[N:])
    assert (zrows >= WBASE[1]).all() and (zrows < WROW).all()

    # Edge list in new ids, with self-loops appended, sorted by destination.
    src_new = newid[src]
    dst_new = newid[dst]
    S_all = np.concatenate([src_new, np.arange(NTOT, dtype=np.int64)])
    T_all = np.concatenate([dst_new, np.arange(NTOT, dtype=np.int64)])
    order_e = np.argsort(T_all, kind="stable")
    S = S_all[order_e]
    T = T_all[order_e]
    E2 = len(S)
    counts = np.bincount(T_all, minlength=NTOT)
    starts = np.zeros(NTOT + 1, dtype=np.int64)
    np.cumsum(counts, out=starts[1:])
    T_start = starts[T]

    # Window assignment: rows < 17408 must use window 0, rows >= 32768 must
    # use window 1; rows in the overlap go to whichever bucket balances the
    # destination's two counts (padded per-tile max is what descriptors cost).
    forced_lo = S < WBASE[1]
    forced_hi = S >= WROW
    flex = ~forced_lo & ~forced_hi
    a_cnt = np.bincount(T[forced_lo], minlength=NTOT)
    c_cnt = np.bincount(T[flex], minlength=NTOT)
    x_cnt = np.clip((counts + 1) // 2 - a_cnt, 0, c_cnt)   # flex edges -> lo
    excl_f = np.cumsum(flex) - flex
    flex_rank = excl_f - excl_f[T_start]
    is_lo = forced_lo | (flex & (flex_rank < x_cnt[T]))
    excl = np.cumsum(is_lo) - is_lo
    lo_rank = excl - excl[T_start]
    hi_rank = (np.arange(E2) - T_start) - lo_rank
    wcnt = [np.bincount(T[is_lo], minlength=NTOT)]
    wcnt.append(counts - wcnt[0])
    wrank = [lo_rank, hi_rank]
    wsel = [is_lo, ~is_lo]

    # per-group per-window uniform slot counts (max over group tiles x cores)
    groups = _groups()
    per_tile = [w.reshape(NCORES, TPC, P).max(axis=(0, 2)) for w in wcnt]
    DG = [[int(per_tile[w][tiles].max()) for tiles in groups] for w in range(2)]

    mats = []
    ii = np.arange(NTOT)[:, None]
    for w in range(2):
        dmax = max(1, max(DG[w]))
        jw = np.arange(dmax)[None, :]
        m = (zrows[(ii * 7 + jw) % len(zrows)] - WBASE[w]).astype(np.int32)
        m[T[wsel[w]], wrank[w][wsel[w]]] = S[wsel[w]] - WBASE[w]
        mats.append(m)

    # Permuted per-node data
    dinv_new = np.zeros(NTOT, dtype=np.float32)
    dinv_new[newid] = dinv
    x_new = np.zeros((NTOT, IN_DIM), dtype=np.float32)
    x_new[newid] = x * dinv[:, None]                 # pre-scaled by dinv
    cnts = np.bincount(batch, minlength=G).astype(np.float32)
    inv_cnt = 1.0 / np.maximum(cnts, 1.0)
    pool_new = np.zeros((NTOT, G), dtype=np.float32)
    pool_new[newid, batch] = inv_cnt[batch]

    W1 = np.ascontiguousarray(np.asarray(W1, NPT))
    W2 = np.ascontiguousarray(np.asarray(W2, NPT))
    Wc1 = np.ascontiguousarray(np.asarray(Wc1, np.float32))
    Wc2 = np.ascontiguousarray(np.asarray(Wc2, np.float32))
    b1r = np.ascontiguousarray(np.broadcast_to(np.asarray(b1, np.float32), (P, 64)))
    b2r = np.ascontiguousarray(np.broadcast_to(np.asarray(b2, np.float32), (P, 64)))
    bc1c = np.ascontiguousarray(np.asarray(bc1, np.float32).reshape(32, 1))
    bc2c = np.ascontiguousarray(np.asarray(bc2, np.float32).reshape(1, 1))

    in_maps = []
    for c in range(NCORES):
        rows = slice(c * NPC, (c + 1) * NPC)
        blocks = []
        for gi, tiles in enumerate(groups):
            for w in range(2):
                D = DG[w][gi]
                vals = np.concatenate(
                    [
                        mats[w][c * NPC + k * P : c * NPC + (k + 1) * P, :D].T.ravel()
                        for k in tiles
                    ]
                )
                b = _pack16(vals)                     # [128, len(tiles)*8*D]
                padc = _pad_to(b.shape[1], ALIGN) - b.shape[1]
                if padc:
                    b = np.concatenate([b, np.zeros((P, padc), np.int16)], axis=1)
                blocks.append(b)
        idx16_c = np.ascontiguousarray(np.concatenate(blocks, axis=1))
        in_maps.append(
            {
                "xt": np.ascontiguousarray(x_new[rows].T.astype(NPT)),  # [128, NPC]
                "idx16": idx16_c,                                # [128, IDXW]
                "dinv": np.ascontiguousarray(dinv_new[rows]),    # [NPC]
                "pool": np.ascontiguousarray(pool_new[rows].astype(NPT)),  # [NPC, G]
                "w1": W1,
                "w2": W2,
                "wc1": Wc1,
                "wc2": Wc2,
                "b1r": b1r,
                "b2r": b2r,
                "bc1": bc1c,
                "bc2": bc2c,
            }
        )
    return DG, in_maps


def build_nc(DG, debug=False, nrep=1, nq=4, table_mode="shared", scratch=32768):
    """Build the SPMD Bass program (shared by all 8 cores)."""
    assert not (debug and nrep > 1)
    groups = _groups()
    idx_off = []
    off = 0
    for gi, tiles in enumerate(groups):
        offs = []
        for w in range(2):
            offs.append(off)
            off += _pad_to(len(tiles) * 8 * DG[w][gi], ALIGN)
        idx_off.append(offs)
    idxw = off

    nc = bacc.Bacc(num_swdge_queues=nq, dynamic_dma_scratch_size=scratch)

    xt = nc.declare_dram_parameter("xt", [IN_DIM, NPC], ADT, isOutput=False)
    idx16 = nc.declare_dram_parameter("idx16", [P, idxw], I16, isOutput=False)
    dinv = nc.declare_dram_parameter("dinv", [NPC], F32, isOutput=False)
    pool = nc.declare_dram_parameter("pool", [NPC, G], ADT, isOutput=False)
    w1 = nc.declare_dram_parameter("w1", [IN_DIM, 64], ADT, isOutput=False)
    w2 = nc.declare_dram_parameter("w2", [64, 64], ADT, isOutput=False)
    wc1 = nc.declare_dram_parameter("wc1", [64, 32], F32, isOutput=False)
    wc2 = nc.declare_dram_parameter("wc2", [32, 1], F32, isOutput=False)
    b1r = nc.declare_dram_parameter("b1r", [P, 64], F32, isOutput=False)
    b2r = nc.declare_dram_parameter("b2r", [P, 64], F32, isOutput=False)
    bc1 = nc.declare_dram_parameter("bc1", [32, 1], F32, isOutput=False)
    bc2 = nc.declare_dram_parameter("bc2", [1, 1], F32, isOutput=False)
    out_ext = nc.declare_dram_parameter("out", [1, G], F32, isOutput=True)

    t1_shard = nc.dram_tensor("t1_shard", [NPC, 64], TDT)
    t2_shard = nc.dram_tensor("t2_shard", [NPC, 64], TDT)
    out_space = "Local" if table_mode == "local_out" else "Shared"
    t1_full = nc.dram_tensor("t1_full", [NTOT, 64], TDT, addr_space=out_space)
    t2_full = nc.dram_tensor("t2_full", [NTOT, 64], TDT, addr_space=out_space)
    gpart = nc.dram_tensor("gpart", [64, G], F32)
    gfull = nc.dram_tensor("gfull", [64, G], F32, addr_space="Shared")

    rg = [list(range(NCORES))]
    relu = mybir.ActivationFunctionType.Relu
    ident_f = mybir.ActivationFunctionType.Identity

    with tile.TileContext(nc) as tc:
        with (
            tc.tile_pool(name="singles", bufs=1) as singles,
            tc.tile_pool(name="resident", bufs=1) as resident,
            tc.tile_pool(name="work", bufs=6) as work,
            tc.tile_pool(name="gath", bufs=3) as gath,
            tc.tile_pool(name="psum", bufs=2, space="PSUM") as psum,
            tc.tile_pool(name="psum1", bufs=1, space="PSUM") as psum1,
        ):
            # --- constants ---
            xt_sb = singles.tile([P, NPC], ADT)
            nc.sync.dma_start(out=xt_sb[:], in_=xt[:])
            idx_sb = singles.tile([P, idxw], I16)
            nc.sync.dma_start(out=idx_sb[:], in_=idx16[:])
            w1_sb = singles.tile([IN_DIM, 64], ADT)
            nc.sync.dma_start(out=w1_sb[:], in_=w1[:])
            w2_sb = singles.tile([64, 64], ADT)
            nc.sync.dma_start(out=w2_sb[:], in_=w2[:])
            wc1_sb = singles.tile([64, 32], F32)
            nc.sync.dma_start(out=wc1_sb[:], in_=wc1[:])
            wc2_sb = singles.tile([32, 1], F32)
            nc.sync.dma_start(out=wc2_sb[:], in_=wc2[:])
            b1r_sb = singles.tile([P, 64], F32)
            nc.sync.dma_start(out=b1r_sb[:], in_=b1r[:])
            b2r_sb = singles.tile([P, 64], F32)
            nc.sync.dma_start(out=b2r_sb[:], in_=b2r[:])
            bc1_sb = singles.tile([32, 1], F32)
            nc.sync.dma_start(out=bc1_sb[:], in_=bc1[:])
            bc2_sb = singles.tile([1, 1], F32)
            nc.sync.dma_start(out=bc2_sb[:], in_=bc2[:])
            ident_sb = singles.tile([P, P], ADT)
            make_identity(nc, ident_sb[:])

            dinv_sb = []
            pool_sb = []
            for k in range(TPC):
                dt = resident.tile([P, 1], F32, tag=f"dinv{k}")
                nc.sync.dma_start(out=dt[:], in_=dinv[k * P : (k + 1) * P, None])
                dinv_sb.append(dt)
                pt = resident.tile([P, G], ADT, tag=f"pool{k}")
                nc.sync.dma_start(out=pt[:], in_=pool[k * P : (k + 1) * P, :])
                pool_sb.append(pt)

            gcnt = [0]  # global SWDGE gather counter -> DMASW lane rotation

            def gather_group(g_t, table, gi, ntile):
                """Two gathers (one per window) covering the whole group."""
                col = 0
                for w in range(2):
                    n = ntile * DG[w][gi]
                    if n == 0:
                        continue
                    o = idx_off[gi][w]
                    # Tile locks each DMASW sem lane (8, round-robin in
                    # creation order) to a single SWDGE queue; follow that.
                    q = (gcnt[0] % 8) // (8 // nq)
                    gcnt[0] += 1
                    nc.gpsimd.dma_gather(
                        out_ap=g_t[:, col * 64 : (col + n) * 64].rearrange(
                            "p (c f) -> p c f", f=64
                        ),
                        in_ap=table[WBASE[w] : WBASE[w] + WROW, :],
                        idxs_ap=idx_sb[:, o : o + 8 * n],
                        num_idxs=P * n,
                        num_idxs_reg=P * n,
                        elem_size=64,
                        single_packet=False,
                        queue_num=q,
                    )
                    col += n

            def aggregate(table, bias_sb):
                """Gather+reduce+scale+bias for one layer; yields (k, pre).

                One tensor_reduce per (group, window) via a 4D AP
                [p, tile, feat, slot] reducing the innermost slot axis."""
                for gi, tiles in enumerate(groups):
                    ntile = len(tiles)
                    D0, D1 = DG[0][gi], DG[1][gi]
                    g_t = gath.tile([P, ntile * (D0 + D1) * 64], TDT, tag="g")
                    gather_group(g_t, table, gi, ntile)
                    s_t = work.tile([P, ntile * 64], F32, tag="s")
                    nc.vector.tensor_reduce(
                        out=s_t[:].rearrange("p (t f) -> p t f", f=64),
                        in_=g_t[:, : ntile * D0 * 64].rearrange(
                            "p (t d f) -> p t f d", d=D0, f=64
                        ),
                        axis=mybir.AxisListType.X,
                        op=mybir.AluOpType.add,
                    )
                    if D1:
                        s2_t = work.tile([P, ntile * 64], F32, tag="s2")
                        nc.vector.tensor_reduce(
                            out=s2_t[:].rearrange("p (t f) -> p t f", f=64),
                            in_=g_t[:, ntile * D0 * 64 :].rearrange(
                                "p (t d f) -> p t f d", d=D1, f=64
                            ),
                            axis=mybir.AxisListType.X,
                            op=mybir.AluOpType.add,
                        )
                        nc.vector.tensor_add(out=s_t[:], in0=s_t[:], in1=s2_t[:])
                    for j, k in enumerate(tiles):
                        pre_t = work.tile([P, 64], F32, tag="pre")
                        nc.vector.scalar_tensor_tensor(
                            out=pre_t[:],
                            in0=s_t[:, j * 64 : (j + 1) * 64],
                            scalar=dinv_sb[k][:],
                            in1=bias_sb[:],
                            op0=mybir.AluOpType.mult,
                            op1=mybir.AluOpType.add,
                        )
                        yield k, pre_t

            for _rep in range(nrep):
                # --- phase B: T1 shard = (x*dinv) @ W1 ---
                for k in range(TPC):
                    ps = psum.tile([P, 64], F32, tag="mm")
                    nc.tensor.matmul(
                        out=ps[:],
                        lhsT=xt_sb[:, k * P : (k + 1) * P],
                        rhs=w1_sb[:],
                        start=True,
                        stop=True,
                    )
                    t1t = work.tile([P, 64], TDT, tag="t1t")
                    nc.scalar.copy(out=t1t[:], in_=ps[:])
                    nc.sync.dma_start(out=t1_shard[k * P : (k + 1) * P, :], in_=t1t[:])

                # --- phase C: AllGather T1 ---
                nc.gpsimd.collective_compute(
                    "AllGather",
                    mybir.AluOpType.bypass,
                    replica_groups=rg,
                    ins=[t1_shard[:]],
                    outs=[t1_full[:]],
                )

                # --- phase D: layer-1 aggregate + T2 shard ---
                for k, pre_t in aggregate(t1_full, b1r_sb):
                    # a1_scaled = dinv*relu(pre) = relu(dinv*pre)   (dinv >= 0)
                    a1s_t = work.tile([P, 64], ADT, tag="a1s")
                    nc.scalar.activation(
                        out=a1s_t[:], in_=pre_t[:], func=relu, scale=dinv_sb[k][:]
                    )
                    pst = psum.tile([64, P], ADT, tag="tr")
                    nc.tensor.transpose(out=pst[:], in_=a1s_t[:], identity=ident_sb[:])
                    a1sT = work.tile([64, P], ADT, tag="a1sT")
                    nc.scalar.copy(out=a1sT[:], in_=pst[:])
                    ps2 = psum.tile([P, 64], F32, tag="mm")
                    nc.tensor.matmul(
                        out=ps2[:], lhsT=a1sT[:], rhs=w2_sb[:], start=True, stop=True
                    )
                    t2t = work.tile([P, 64], TDT, tag="t2t")
                    nc.scalar.copy(out=t2t[:], in_=ps2[:])
                    nc.sync.dma_start(out=t2_shard[k * P : (k + 1) * P, :], in_=t2t[:])

                # --- phase E: AllGather T2 ---
                nc.gpsimd.collective_compute(
                    "AllGather",
                    mybir.AluOpType.bypass,
                    replica_groups=rg,
                    ins=[t2_shard[:]],
                    outs=[t2_full[:]],
                )

                # --- phase F: layer-2 aggregate + pooling matmul ---
                pool_ps = psum1.tile([64, G], F32)
                for k, pre_t in aggregate(t2_full, b2r_sb):
                    a2_t = work.tile([P, 64], ADT, tag="a2")
                    nc.scalar.activation(out=a2_t[:], in_=pre_t[:], func=relu)
                    nc.tensor.matmul(
                        out=pool_ps[:],
                        lhsT=a2_t[:],
                        rhs=pool_sb[k][:],
                        start=(k == 0),
                        stop=(k == TPC - 1),
                    )

                # --- phase G: AllReduce pooled partials ---
                gpart_sb = work.tile([64, G], F32, tag="gp")
                nc.scalar.copy(out=gpart_sb[:], in_=pool_ps[:])
                nc.sync.dma_start(out=gpart[:], in_=gpart_sb[:])
                nc.gpsimd.collective_compute(
                    "AllReduce",
                    mybir.AluOpType.add,
                    replica_groups=rg,
                    ins=[gpart[:]],
                    outs=[gfull[:]],
                )
                gsum_sb = work.tile([64, G], F32, tag="gs")
                nc.sync.dma_start(out=gsum_sb[:], in_=gfull[:])

                # --- phase H: MLP head (replicated) ---
                ps_z = psum1.tile([32, G], F32, tag="hz")
                nc.tensor.matmul(
                    out=ps_z[:], lhsT=wc1_sb[:], rhs=gsum_sb[:], start=True, stop=True
                )
                z_sb = work.tile([32, G], F32, tag="z")
                nc.scalar.activation(out=z_sb[:], in_=ps_z[:], func=relu, bias=bc1_sb[:])
                ps_o = psum1.tile([1, G], F32, tag="ho")
                nc.tensor.matmul(
                    out=ps_o[:], lhsT=wc2_sb[:], rhs=z_sb[:], start=True, stop=True
                )
                o_sb = work.tile([1, G], F32, tag="o")
                nc.scalar.activation(
                    out=o_sb[:], in_=ps_o[:], func=ident_f, bias=bc2_sb[:]
                )
                nc.sync.dma_start(out=out_ext[:], in_=o_sb[:])

            if debug:
                t1_dump = nc.declare_dram_parameter(
                    "t1_dump", [NTOT, 64], TDT, isOutput=True
                )
                t2_dump = nc.declare_dram_parameter(
                    "t2_dump", [NTOT, 64], TDT, isOutput=True
                )
                nc.sync.dma_start(out=t1_dump[:], in_=t1_full[:])
                nc.sync.dma_start(out=t2_dump[:], in_=t2_full[:])

    nc.finalize()
    return nc


def kernel(**inputs):
    DG, in_maps = prep(**inputs)
    nc = build_nc(DG)
    res = run_bass_kernel_spmd(nc, in_maps, list(range(NCORES)))
    out = np.asarray(res.results[0]["out"], dtype=np.float32).reshape(G, 1)
    return out



# revision 24
# speedup vs baseline: 8.1869x; 1.8446x over previous
"""2-layer GCN + global mean-pool + MLP head on 8 TRN2 NeuronCores.

Algorithm (matches the PyG-style reference):
    h1 = relu(Ahat @ (x @ W1) + b1)
    h2 = relu(Ahat @ (h1 @ W2) + b2)
    g  = segment_mean(h2, batch)          # [64, hid]
    out = relu(g @ Wc1 + bc1) @ Wc2 + bc2 # [64, 1]
with Ahat = D^-1/2 (A + I) D^-1/2 (in-degree based, self-loops added).

Key factorization: the edge weight dinv[s]*dinv[d] splits per-node, so we
pre-scale "table" rows by dinv (T = dinv * (h @ W)) and aggregation becomes a
pure gather+sum followed by a per-destination dinv scale:
    out[d] = dinv[d] * sum_{s in N(d) ∪ {d}} T[s]  + b

Distribution: nodes are permuted so that similar-(in)degree nodes share a
128-row tile; 49 tiles per core x 8 cores = 50176 padded rows.  Each core:
  1. computes T1 for its rows via TensorE (x pre-scaled/transposed on host),
  2. AllGather -> full T1 table in every core's HBM,
  3. per tile group: bulk dma_gather of all slot rows (slot-major layout),
     DVE sums over slots, scale/bias/relu, matmul with W2 -> T2 shard,
  4. AllGather T2, second aggregate pass, pooling matmul into PSUM,
  5. AllReduce pooled [hid, graphs] partials, replicated tiny MLP head.

dma_gather (InstDMAGatherAnt) takes int16 indices, so the 50176-row table is
addressed through two overlapping 32768-row windows ([0, 32768) and
[17408, 50176)); each node's slots are bucketed per window (rows in the
overlap go to whichever bucket balances the two), padded per tile to the
group bucket max, with pad slots rotating over dummy rows in the overlap
region (T value exactly 0, and spread to avoid HBM hot lines).  TGROUP tiles
share one gather instruction per window to amortize per-instruction cost,
and instructions round-robin over 4 SWDGE queues.
"""

import numpy as np

import concourse.bass as bass
import concourse.mybir as mybir
import concourse.tile as tile
from concourse import bacc
from concourse.bass_utils import run_bass_kernel_spmd
from concourse.masks import make_identity

NCORES = 8
P = 128
N = 50000
G = 64
IN_DIM = 128
TPC = 49                 # node tiles per core
NPC = TPC * P            # 6272 rows per core
NTOT = NPC * NCORES      # 50176
WROW = 32768             # int16 index window width
WBASE = (0, NTOT - WROW)  # two overlapping gather windows
TGROUP = 1               # tiles per gather instruction
ALIGN = 256              # idx block column alignment (x2B = 512B)

F32 = mybir.dt.float32
F16 = mybir.dt.float16
I16 = mybir.dt.int16
TDT = F32                # gather-table dtype: dma_gather needs 256B rows -> f32
ADT = F32                # activation/weight dtype (PE inputs, pool matmul)
NPT = np.float32


def _pad_to(x, a):
    return (x + a - 1) // a * a


def _groups():
    return [list(range(g, min(g + TGROUP, TPC))) for g in range(0, TPC, TGROUP)]


def _pack16(vals):
    """int16 index layout consumed by the Q7 dma_gather kernel:
    sbuf[p, i] = vals[i*16 + p%16], replicated across the 8 partition groups."""
    m = len(vals) // 16
    a = vals.astype(np.int16).reshape(m, 16).T      # [16, m]
    return np.tile(a, (8, 1))                        # [128, m]


def prep(x, edge_index, batch, W1, b1, W2, b2, Wc1, bc1, Wc2, bc2):
    """Host-side graph preprocessing -> (DG, in_maps).

    DG[w][gi]: per-window slots per tile for gather group gi (uniform over
    the group's tiles and all cores).
    """
    x = np.ascontiguousarray(np.asarray(x, dtype=np.float32))
    ei = np.asarray(edge_index).astype(np.int64)
    batch = np.asarray(batch).astype(np.int64)
    src, dst = ei[0], ei[1]

    indeg = np.bincount(dst, minlength=N)
    d_slots = (indeg + 1).astype(np.int64)          # incident slots incl. self
    dinv = (1.0 / np.sqrt(d_slots.astype(np.float32))).astype(np.float32)

    # Sort nodes by slot count desc; tiles of 128 consecutive sorted nodes
    # have near-uniform degree.  Tile t -> core t%8, tile position t//8.
    order = np.argsort(-d_slots, kind="stable")
    pos = np.arange(NTOT)
    t_idx = pos // P
    newrow = (t_idx % NCORES) * NPC + (t_idx // NCORES) * P + pos % P
    # Dummy (padding) sorted positions are N..NTOT-1.  Gather padding targets
    # dummy rows (their T value is exactly 0).  Move every dummy into the
    # region covered by both windows ([17408, 32768)) by swapping with
    # similar-(low-)degree real nodes there, and spread pads over all of them
    # so pad reads don't hot-spot a single HBM line.
    ndum = NTOT - N
    dummy_pos = np.arange(N, NTOT)
    in_overlap = (newrow >= WBASE[1]) & (newrow < WROW)
    cand = np.where(in_overlap[:N])[0][-ndum:]    # lowest-degree reals there
    assert len(cand) == ndum
    tmp = newrow[cand].copy()
    newrow[cand] = newrow[dummy_pos]
    newrow[dummy_pos] = tmp
    newid = np.empty(N, dtype=np.int64)
    newid[order] = newrow[:N]
    zrows = np.sort(newrow[N:])
    assert (zrows >= WBASE[1]).all() and (zrows < WROW).all()

    # Edge list in new ids, with self-loops appended, sorted by destination.
    src_new = newid[src]
    dst_new = newid[dst]
    S_all = np.concatenate([src_new, np.arange(NTOT, dtype=np.int64)])
    T_all = np.concatenate([dst_new, np.arange(NTOT, dtype=np.int64)])
    order_e = np.argsort(T_all, kind="stable")
    S = S_all[order_e]
    T = T_all[order_e]
    E2 = len(S)
    counts = np.bincount(T_all, minlength=NTOT)
    starts = np.zeros(NTOT + 1, dtype=np.int64)
    np.cumsum(counts, out=starts[1:])
    T_start = starts[T]

    # Window assignment: rows < 17408 must use window 0, rows >= 32768 must
    # use window 1; rows in the overlap go to whichever bucket balances the
    # destination's two counts (padded per-tile max is what descriptors cost).
    forced_lo = S < WBASE[1]
    forced_hi = S >= WROW
    flex = ~forced_lo & ~forced_hi
    a_cnt = np.bincount(T[forced_lo], minlength=NTOT)
    c_cnt = np.bincount(T[flex], minlength=NTOT)
    x_cnt = np.clip((counts + 1) // 2 - a_cnt, 0, c_cnt)   # flex edges -> lo
    excl_f = np.cumsum(flex) - flex
    flex_rank = excl_f - excl_f[T_start]
    is_lo = forced_lo | (flex & (flex_rank < x_cnt[T]))
    excl = np.cumsum(is_lo) - is_lo
    lo_rank = excl - excl[T_start]
    hi_rank = (np.arange(E2) - T_start) - lo_rank
    wcnt = [np.bincount(T[is_lo], minlength=NTOT)]
    wcnt.append(counts - wcnt[0])
    wrank = [lo_rank, hi_rank]
    wsel = [is_lo, ~is_lo]

    # per-group per-window uniform slot counts (max over group tiles x cores)
    groups = _groups()
    per_tile = [w.reshape(NCORES, TPC, P).max(axis=(0, 2)) for w in wcnt]
    DG = [[int(per_tile[w][tiles].max()) for tiles in groups] for w in range(2)]

    mats = []
    ii = np.arange(NTOT)[:, None]
    for w in range(2):
        dmax = max(1, max(DG[w]))
        jw = np.arange(dmax)[None, :]
        m = (zrows[(ii * 7 + jw) % len(zrows)] - WBASE[w]).astype(np.int32)
        m[T[wsel[w]], wrank[w][wsel[w]]] = S[wsel[w]] - WBASE[w]
        mats.append(m)

    # Permuted per-node data
    dinv_new = np.zeros(NTOT, dtype=np.float32)
    dinv_new[newid] = dinv
    x_new = np.zeros((NTOT, IN_DIM), dtype=np.float32)
    x_new[newid] = x * dinv[:, None]                 # pre-scaled by dinv
    cnts = np.bincount(batch, minlength=G).astype(np.float32)
    inv_cnt = 1.0 / np.maximum(cnts, 1.0)
    pool_new = np.zeros((NTOT, G), dtype=np.float32)
    pool_new[newid, batch] = inv_cnt[batch]

    W1 = np.ascontiguousarray(np.asarray(W1, NPT))
    W2 = np.ascontiguousarray(np.asarray(W2, NPT))
    Wc1 = np.ascontiguousarray(np.asarray(Wc1, np.float32))
    Wc2 = np.ascontiguousarray(np.asarray(Wc2, np.float32))
    b1r = np.ascontiguousarray(np.broadcast_to(np.asarray(b1, np.float32), (P, 64)))
    b2r = np.ascontiguousarray(np.broadcast_to(np.asarray(b2, np.float32), (P, 64)))
    bc1c = np.ascontiguousarray(np.asarray(bc1, np.float32).reshape(32, 1))
    bc2c = np.ascontiguousarray(np.asarray(bc2, np.float32).reshape(1, 1))

    in_maps = []
    for c in range(NCORES):
        rows = slice(c * NPC, (c + 1) * NPC)
        blocks = []
        for gi, tiles in enumerate(groups):
            for w in range(2):
                D = DG[w][gi]
                vals = np.concatenate(
                    [
                        mats[w][c * NPC + k * P : c * NPC + (k + 1) * P, :D].T.ravel()
                        for k in tiles
                    ]
                )
                b = _pack16(vals)                     # [128, len(tiles)*8*D]
                padc = _pad_to(b.shape[1], ALIGN) - b.shape[1]
                if padc:
                    b = np.concatenate([b, np.zeros((P, padc), np.int16)], axis=1)
                blocks.append(b)
        idx16_c = np.ascontiguousarray(np.concatenate(blocks, axis=1))
        in_maps.append(
            {
                "xt": np.ascontiguousarray(x_new[rows].T.astype(NPT)),  # [128, NPC]
                "idx16": idx16_c,                                # [128, IDXW]
                "dinv": np.ascontiguousarray(dinv_new[rows]),    # [NPC]
                "pool": np.ascontiguousarray(pool_new[rows].astype(NPT)),  # [NPC, G]
                "w1": W1,
                "w2": W2,
                "wc1": Wc1,
                "wc2": Wc2,
                "b1r": b1r,
                "b2r": b2r,
                "bc1": bc1c,
                "bc2": bc2c,
            }
        )
    return DG, in_maps


def build_nc(DG, debug=False, nrep=1, nq=4, table_mode="shared", scratch=32768):
    """Build the SPMD Bass program (shared by all 8 cores)."""
    assert not (debug and nrep > 1)
    groups = _groups()
    idx_off = []
    off = 0
    for gi, tiles in enumerate(groups):
        offs = []
        for w in range(2):
            offs.append(off)
            off += _pad_to(len(tiles) * 8 * DG[w][gi], ALIGN)
        idx_off.append(offs)
    idxw = off

    nc = bacc.Bacc(num_swdge_queues=nq, dynamic_dma_scratch_size=scratch)

    xt = nc.declare_dram_parameter("xt", [IN_DIM, NPC], ADT, isOutput=False)
    idx16 = nc.declare_dram_parameter("idx16", [P, idxw], I16, isOutput=False)
    dinv = nc.declare_dram_parameter("dinv", [NPC], F32, isOutput=False)
    pool = nc.declare_dram_parameter("pool", [NPC, G], ADT, isOutput=False)
    w1 = nc.declare_dram_parameter("w1", [IN_DIM, 64], ADT, isOutput=False)
    w2 = nc.declare_dram_parameter("w2", [64, 64], ADT, isOutput=False)
    wc1 = nc.declare_dram_parameter("wc1", [64, 32], F32, isOutput=False)
    wc2 = nc.declare_dram_parameter("wc2", [32, 1], F32, isOutput=False)
    b1r = nc.declare_dram_parameter("b1r", [P, 64], F32, isOutput=False)
    b2r = nc.declare_dram_parameter("b2r", [P, 64], F32, isOutput=False)
    bc1 = nc.declare_dram_parameter("bc1", [32, 1], F32, isOutput=False)
    bc2 = nc.declare_dram_parameter("bc2", [1, 1], F32, isOutput=False)
    out_ext = nc.declare_dram_parameter("out", [1, G], F32, isOutput=True)

    t1_shard = nc.dram_tensor("t1_shard", [NPC, 64], TDT)
    t2_shard = nc.dram_tensor("t2_shard", [NPC, 64], TDT)
    out_space = "Local" if table_mode == "local_out" else "Shared"
    t1_full = nc.dram_tensor("t1_full", [NTOT, 64], TDT, addr_space=out_space)
    t2_full = nc.dram_tensor("t2_full", [NTOT, 64], TDT, addr_space=out_space)
    gpart = nc.dram_tensor("gpart", [64, G], F32)
    gfull = nc.dram_tensor("gfull", [64, G], F32, addr_space="Shared")

    rg = [list(range(NCORES))]
    relu = mybir.ActivationFunctionType.Relu
    ident_f = mybir.ActivationFunctionType.Identity

    with tile.TileContext(nc) as tc:
        with (
            tc.tile_pool(name="singles", bufs=1) as singles,
            tc.tile_pool(name="resident", bufs=1) as resident,
            tc.tile_pool(name="work", bufs=6) as work,
            tc.tile_pool(name="gath", bufs=8) as gath,
            tc.tile_pool(name="psum", bufs=2, space="PSUM") as psum,
            tc.tile_pool(name="psum1", bufs=1, space="PSUM") as psum1,
        ):
            # --- constants ---
            xt_sb = singles.tile([P, NPC], ADT)
            nc.sync.dma_start(out=xt_sb[:], in_=xt[:])
            idx_sb = singles.tile([P, idxw], I16)
            nc.sync.dma_start(out=idx_sb[:], in_=idx16[:])
            w1_sb = singles.tile([IN_DIM, 64], ADT)
            nc.sync.dma_start(out=w1_sb[:], in_=w1[:])
            w2_sb = singles.tile([64, 64], ADT)
            nc.sync.dma_start(out=w2_sb[:], in_=w2[:])
            wc1_sb = singles.tile([64, 32], F32)
            nc.sync.dma_start(out=wc1_sb[:], in_=wc1[:])
            wc2_sb = singles.tile([32, 1], F32)
            nc.sync.dma_start(out=wc2_sb[:], in_=wc2[:])
            b1r_sb = singles.tile([P, 64], F32)
            nc.sync.dma_start(out=b1r_sb[:], in_=b1r[:])
            b2r_sb = singles.tile([P, 64], F32)
            nc.sync.dma_start(out=b2r_sb[:], in_=b2r[:])
            bc1_sb = singles.tile([32, 1], F32)
            nc.sync.dma_start(out=bc1_sb[:], in_=bc1[:])
            bc2_sb = singles.tile([1, 1], F32)
            nc.sync.dma_start(out=bc2_sb[:], in_=bc2[:])
            ident_sb = singles.tile([P, P], ADT)
            make_identity(nc, ident_sb[:])

            dinv_sb = []
            pool_sb = []
            for k in range(TPC):
                dt = resident.tile([P, 1], F32, tag=f"dinv{k}")
                nc.sync.dma_start(out=dt[:], in_=dinv[k * P : (k + 1) * P, None])
                dinv_sb.append(dt)
                pt = resident.tile([P, G], ADT, tag=f"pool{k}")
                nc.sync.dma_start(out=pt[:], in_=pool[k * P : (k + 1) * P, :])
                pool_sb.append(pt)

            gcnt = [0]  # global SWDGE gather counter -> DMASW lane rotation

            def gather_group(g_t, table, gi, ntile):
                """Two gathers (one per window) covering the whole group."""
                col = 0
                for w in range(2):
                    n = ntile * DG[w][gi]
                    if n == 0:
                        continue
                    o = idx_off[gi][w]
                    # Tile locks each DMASW sem lane (8, round-robin in
                    # creation order) to a single SWDGE queue; follow that.
                    q = (gcnt[0] % 8) // (8 // nq)
                    gcnt[0] += 1
                    nc.gpsimd.dma_gather(
                        out_ap=g_t[:, col * 64 : (col + n) * 64].rearrange(
                            "p (c f) -> p c f", f=64
                        ),
                        in_ap=table[WBASE[w] : WBASE[w] + WROW, :],
                        idxs_ap=idx_sb[:, o : o + 8 * n],
                        num_idxs=P * n,
                        num_idxs_reg=P * n,
                        elem_size=64,
                        single_packet=False,
                        queue_num=q,
                    )
                    col += n

            def aggregate(table, bias_sb):
                """Gather+reduce+scale+bias for one layer; yields (k, pre).

                One tensor_reduce per (group, window) via a 4D AP
                [p, tile, feat, slot] reducing the innermost slot axis."""
                for gi, tiles in enumerate(groups):
                    ntile = len(tiles)
                    D0, D1 = DG[0][gi], DG[1][gi]
                    g_t = gath.tile([P, ntile * (D0 + D1) * 64], TDT, tag="g")
                    gather_group(g_t, table, gi, ntile)
                    s_t = work.tile([P, ntile * 64], F32, tag="s")
                    nc.vector.tensor_reduce(
                        out=s_t[:].rearrange("p (t f) -> p t f", f=64),
                        in_=g_t[:, : ntile * D0 * 64].rearrange(
                            "p (t d f) -> p t f d", d=D0, f=64
                        ),
                        axis=mybir.AxisListType.X,
                        op=mybir.AluOpType.add,
                    )
                    if D1:
                        s2_t = work.tile([P, ntile * 64], F32, tag="s2")
                        nc.vector.tensor_reduce(
                            out=s2_t[:].rearrange("p (t f) -> p t f", f=64),
                            in_=g_t[:, ntile * D0 * 64 :].rearrange(
                                "p (t d f) -> p t f d", d=D1, f=64
                            ),
                            axis=mybir.AxisListType.X,
                            op=mybir.AluOpType.add,
                        )
                        nc.vector.tensor_add(out=s_t[:], in0=s_t[:], in1=s2_t[:])
                    for j, k in enumerate(tiles):
                        pre_t = work.tile([P, 64], F32, tag="pre")
                        nc.vector.scalar_tensor_tensor(
                            out=pre_t[:],
                            in0=s_t[:, j * 64 : (j + 1) * 64],
                            scalar=dinv_sb[k][:],
                            in1=bias_sb[:],
                            op0=mybir.AluOpType.mult,
                            op1=mybir.AluOpType.add,
                        )
                        yield k, pre_t

            for _rep in range(nrep):
                # --- phase B: T1 shard = (x*dinv) @ W1 ---
                for k in range(TPC):
                    ps = psum.tile([P, 64], F32, tag="mm")
                    nc.tensor.matmul(
                        out=ps[:],
                        lhsT=xt_sb[:, k * P : (k + 1) * P],
                        rhs=w1_sb[:],
                        start=True,
                        stop=True,
                    )
                    t1t = work.tile([P, 64], TDT, tag="t1t")
                    nc.scalar.copy(out=t1t[:], in_=ps[:])
                    nc.sync.dma_start(out=t1_shard[k * P : (k + 1) * P, :], in_=t1t[:])

                # --- phase C: AllGather T1 ---
                nc.gpsimd.collective_compute(
                    "AllGather",
                    mybir.AluOpType.bypass,
                    replica_groups=rg,
                    ins=[t1_shard[:]],
                    outs=[t1_full[:]],
                )

                # --- phase D: layer-1 aggregate + T2 shard ---
                for k, pre_t in aggregate(t1_full, b1r_sb):
                    # a1_scaled = dinv*relu(pre) = relu(dinv*pre)   (dinv >= 0)
                    a1s_t = work.tile([P, 64], ADT, tag="a1s")
                    nc.scalar.activation(
                        out=a1s_t[:], in_=pre_t[:], func=relu, scale=dinv_sb[k][:]
                    )
                    pst = psum.tile([64, P], F32, tag="tr")
                    nc.tensor.transpose(out=pst[:], in_=a1s_t[:], identity=ident_sb[:])
                    a1sT = work.tile([64, P], ADT, tag="a1sT")
                    nc.scalar.copy(out=a1sT[:], in_=pst[:])
                    ps2 = psum.tile([P, 64], F32, tag="mm")
                    nc.tensor.matmul(
                        out=ps2[:], lhsT=a1sT[:], rhs=w2_sb[:], start=True, stop=True
                    )
                    t2t = work.tile([P, 64], TDT, tag="t2t")
                    nc.scalar.copy(out=t2t[:], in_=ps2[:])
                    nc.sync.dma_start(out=t2_shard[k * P : (k + 1) * P, :], in_=t2t[:])

                # --- phase E: AllGather T2 ---
                nc.gpsimd.collective_compute(
                    "AllGather",
                    mybir.AluOpType.bypass,
                    replica_groups=rg,
                    ins=[t2_shard[:]],
                    outs=[t2_full[:]],
                )

                # --- phase F: layer-2 aggregate + pooling matmul ---
                pool_ps = psum1.tile([64, G], F32)
                for k, pre_t in aggregate(t2_full, b2r_sb):
                    a2_t = work.tile([P, 64], ADT, tag="a2")
                    nc.scalar.activation(out=a2_t[:], in_=pre_t[:], func=relu)
                    nc.tensor.matmul(
                        out=pool_ps[:],
                        lhsT=a2_t[:],
                        rhs=pool_sb[k][:],
                        start=(k == 0),
                        stop=(k == TPC - 1),
                    )

                # --- phase G: AllReduce pooled partials ---
                gpart_sb = work.tile([64, G], F32, tag="gp")
                nc.scalar.copy(out=gpart_sb[:], in_=pool_ps[:])
                nc.sync.dma_start(out=gpart[:], in_=gpart_sb[:])
                nc.gpsimd.collective_compute(
                    "AllReduce",
                    mybir.AluOpType.add,
                    replica_groups=rg,
                    ins=[gpart[:]],
                    outs=[gfull[:]],
                )
                gsum_sb = work.tile([64, G], F32, tag="gs")
                nc.sync.dma_start(out=gsum_sb[:], in_=gfull[:])

                # --- phase H: MLP head (replicated) ---
                ps_z = psum1.tile([32, G], F32, tag="hz")
                nc.tensor.matmul(
                    out=ps_z[:], lhsT=wc1_sb[:], rhs=gsum_sb[:], start=True, stop=True
                )
                z_sb = work.tile([32, G], F32, tag="z")
                nc.scalar.activation(out=z_sb[:], in_=ps_z[:], func=relu, bias=bc1_sb[:])
                ps_o = psum1.tile([1, G], F32, tag="ho")
                nc.tensor.matmul(
                    out=ps_o[:], lhsT=wc2_sb[:], rhs=z_sb[:], start=True, stop=True
                )
                o_sb = work.tile([1, G], F32, tag="o")
                nc.scalar.activation(
                    out=o_sb[:], in_=ps_o[:], func=ident_f, bias=bc2_sb[:]
                )
                nc.sync.dma_start(out=out_ext[:], in_=o_sb[:])

            if debug:
                t1_dump = nc.declare_dram_parameter(
                    "t1_dump", [NTOT, 64], TDT, isOutput=True
                )
                t2_dump = nc.declare_dram_parameter(
                    "t2_dump", [NTOT, 64], TDT, isOutput=True
                )
                nc.sync.dma_start(out=t1_dump[:], in_=t1_full[:])
                nc.sync.dma_start(out=t2_dump[:], in_=t2_full[:])

    nc.finalize()
    return nc


def kernel(**inputs):
    DG, in_maps = prep(**inputs)
    nc = build_nc(DG)
    res = run_bass_kernel_spmd(nc, in_maps, list(range(NCORES)))
    out = np.asarray(res.results[0]["out"], dtype=np.float32).reshape(G, 1)
    return out

